# revision 6
# baseline (speedup 1.0000x reference)
"""Multi-head attention (B=8, N=1024, D=768, H=12, softmax over full dim-scaled
scores) on 8 Trainium2 NeuronCores, data-parallel over the batch dimension:
core b computes batch element b end-to-end; no collectives.

v2 schedule (from trace analysis of the 204 us baseline):
  - Host pre-packs every tensor into its exact SBUF layout so the input
    phase is 7 large DMAs spread over 4 queues (baseline: 43 small DMAs,
    ~8 us of serialized issue).
  - The attention middle phase is ScalarE-bound (96 exp ACTs ~1.11 us each).
    Scores PSUM is one rotating 2-deep pool consumed in strict A/B
    alternation so the exp stream never waits on a PSUM bank; PV groups and
    projection/output-projection filler are interleaved between the scores
    matmuls at sub-group granularity to keep PE warm without delaying them.
  - Softmax denominators: batched reciprocal_approx_fast (51 ULP, 5x faster
    than the exact DVE reciprocal that cost 3.3 us per pair).
  - Output projection pass-1 (bias + ct0..4 of the contraction) runs as PE
    filler inside the pair-5 loop; only the ct5 rank-128 update, the final
    add and the stores remain after the drain. Stores are 8 full-row DMAs
    alternating between two queues.
"""

import numpy as np
import ml_dtypes

import concourse.bass as bass
import concourse.bacc as bacc
import concourse.tile as tile
from concourse import mybir
from concourse.bass_utils import run_bass_kernel_spmd

f32 = mybir.dt.float32
bf16 = mybir.dt.bfloat16

B = 8
N = 1024
D = 768
H = 12
DH = 64
SCALE = float(D) ** -0.5
NT = N // 128   # 8 sequence tiles
KT = D // 128   # 6 feature tiles
NPAIR = H // 2  # 6 head pairs


def build_bass():
    nc = bacc.Bacc("TRN2", target_bir_lowering=False, debug=False, num_devices=B)
    # all inputs pre-packed on host to [partition, kt, col] SBUF layouts
    xa_d = nc.dram_tensor("xa", [128, KT, 512], bf16, kind="ExternalInput")
    xb_d = nc.dram_tensor("xb", [128, KT, 512], bf16, kind="ExternalInput")
    wqk1_d = nc.dram_tensor("wqk1", [128, KT, 256], bf16, kind="ExternalInput")
    wqk2_d = nc.dram_tensor("wqk2", [128, KT, 1280], bf16, kind="ExternalInput")
    wv_d = nc.dram_tensor("wv", [128, KT, D], bf16, kind="ExternalInput")
    wo_d = nc.dram_tensor("wo", [128, KT, D], bf16, kind="ExternalInput")
    bo_d = nc.dram_tensor("bo", [D], f32, kind="ExternalInput")
    out_d = nc.dram_tensor("out", [N, D], f32, kind="ExternalOutput")

    with tile.TileContext(nc) as tc:
        with tc.tile_pool(name="persist", bufs=1) as pp:
            # persistent SBUF tensors
            xT_sb = pp.tile([128, KT, N], bf16)        # x^T feature tiles
            wqk_sb = pp.tile([128, KT, 2 * D], bf16)   # q|k cols, pair-packed
            wv_sb = pp.tile([128, KT, D], bf16)        # v cols (head order)
            wo_sb = pp.tile([128, KT, D], bf16)        # W_out feature tiles
            qkT = pp.tile([128, 2 * KT, N], bf16)      # q,k feature-major
            vaug = pp.tile([128, NT, H, DH + 1], bf16)  # v token-major + ones
            aoT = pp.tile([128, KT, N], bf16)          # attention out, feature-major
            osb_all = pp.tile([128, NT, 2, 384], bf16)  # out-proj pass-1 partials
            bias_f32 = pp.tile([1, D], f32)
            bias_bf = pp.tile([1, D], bf16)
            ones_col = pp.tile([1, 128], bf16)

            # ---- input DMAs: one large transfer per packed tensor, spread
            # over four queues so issue (~0.6 us each) and transfer overlap.
            nc.sync.dma_start(out=xT_sb[:, :, 0:512], in_=xa_d[:, :, :])
            nc.scalar.dma_start(out=wqk_sb[:, :, 0:256], in_=wqk1_d[:, :, :])
            nc.gpsimd.dma_start(out=wv_sb, in_=wv_d[:, :, :])
            nc.sync.dma_start(out=xT_sb[:, :, 512:1024], in_=xb_d[:, :, :])
            nc.scalar.dma_start(out=wqk_sb[:, :, 256:1536], in_=wqk2_d[:, :, :])
            nc.gpsimd.dma_start(out=wo_sb, in_=wo_d[:, :, :])
            bo_ap = bo_d[:]
            nc.sync.dma_start(
                out=bias_f32,
                in_=bass.AP(tensor=bo_ap.tensor, offset=bo_ap.offset,
                            ap=[[0, 1]] + list(bo_ap.ap)),
            )
            nc.vector.tensor_copy(out=bias_bf, in_=bias_f32)
            nc.gpsimd.memset(ones_col, 1.0)
            nc.vector.memset(vaug[:, :, :, DH], 1.0)
            # dummy activation: pulls the exp ACT-table load (~2.7 us) into
            # the initial DMA wait instead of the first real exp
            warm = pp.tile([1, 2], f32)
            nc.vector.memset(warm, 0.0)
            nc.scalar.activation(out=warm, in_=warm,
                                 func=mybir.ActivationFunctionType.Exp)

            with tc.tile_pool(name="sbC", bufs=4) as sbC, \
                 tc.tile_pool(name="sbAug", bufs=5) as sbAug, \
                 tc.tile_pool(name="sbCs", bufs=2) as sbCs, \
                 tc.tile_pool(name="psS", bufs=2, space="PSUM") as psS, \
                 tc.tile_pool(name="psF", bufs=2, space="PSUM") as psF, \
                 tc.tile_pool(name="psV", bufs=2, space="PSUM") as psV:

                def emit_qk_chunk(p, qk, it):
                    # qkT[:, 2p+qk, it-half] = W_{q|k,pair p}^T @ x^T
                    ps = psF.tile([128, 512], f32, tag="fill", name="psqk")
                    for kt in range(KT):
                        nc.tensor.matmul(
                            ps,
                            wqk_sb[:, kt, 256 * p + 128 * qk:256 * p + 128 * (qk + 1)],
                            xT_sb[:, kt, it * 512:(it + 1) * 512],
                            start=(kt == 0), stop=(kt == KT - 1),
                        )
                    nc.vector.tensor_copy(
                        out=qkT[:, 2 * p + qk, it * 512:(it + 1) * 512], in_=ps
                    )

                def emit_v_chunk(jt, et):
                    # v[jt-tile, 6 heads] = x @ W_v  (+ strided head layout)
                    ps = psF.tile([128, 384], f32, tag="fill", name="psv")
                    for kt in range(KT):
                        nc.tensor.matmul(
                            ps,
                            xT_sb[:, kt, jt * 128:(jt + 1) * 128],
                            wv_sb[:, kt, et * 384:(et + 1) * 384],
                            start=(kt == 0), stop=(kt == KT - 1),
                        )
                    nc.vector.tensor_copy(
                        out=vaug[:, jt, 6 * et:6 * (et + 1), 0:DH],
                        in_=ps.rearrange("p (h d) -> p h d", d=DH),
                    )

                p1_cmax = {}

                def emit_pass1_group(nt, et, cmax):
                    # osb_all[nt, et] = bias + sum_{ct<=cmax} aoT_ct^T @ W_out
                    # (cmax limited by which head pairs are normalized yet;
                    # pass 2 adds the rest)
                    p1_cmax[(nt, et)] = cmax
                    ps = psF.tile([128, 384], f32, tag="fill", name="psp1")
                    nc.tensor.matmul(
                        ps, ones_col, bias_bf[:, et * 384:(et + 1) * 384],
                        start=True, stop=False,
                    )
                    for ct in range(cmax + 1):
                        nc.tensor.matmul(
                            ps,
                            aoT[:, ct, nt * 128:(nt + 1) * 128],
                            wo_sb[:, ct, et * 384:(et + 1) * 384],
                            start=False, stop=(ct == cmax),
                        )
                    nc.vector.tensor_copy(out=osb_all[:, nt, et, :], in_=ps)

                def emit_pv_half(q, parity, it, st, half):
                    # softmax-numerator matmul group of pair q, split in two
                    # so a scores matmul can slot between the halves:
                    # out_augT[d+1, i-half] = [v_h | 1]^T @ expT_h
                    e = st["eA"] if parity == 0 else st["eB"]
                    h = 2 * q + parity
                    idx = 2 * parity + it
                    if half == 0:
                        st["pv"][idx] = psV.tile([DH + 1, 512], f32, tag="pv",
                                                 name="pvps")
                    ops = st["pv"][idx]
                    for jt_ in (range(0, 4) if half == 0 else range(4, NT)):
                        nc.tensor.matmul(
                            ops,
                            vaug[:, jt_, h, :],
                            e[:, jt_, it * 512:(it + 1) * 512],
                            start=(jt_ == 0), stop=(jt_ == NT - 1),
                        )
                    if half == 1:
                        aug = sbAug.tile([DH + 1, 512], f32, tag="aug")
                        nc.vector.tensor_copy(out=aug, in_=ops)
                        nc.vector.tensor_copy(
                            out=st["s4"][32 * idx:32 * idx + 1, :],
                            in_=aug[DH:DH + 1, :],
                        )
                        st["augs"][idx] = aug

                def emit_pv_tail(q, st):
                    # batched approx-reciprocal of the pair's 4 softmax
                    # denominators (rows 0/32/64/96 of s4), broadcast, scale.
                    r4 = sbCs.tile([97, 512], f32, tag="r4")
                    nc.vector.reciprocal_approx_fast(out=r4, in_=st["s4"])
                    for parity in (0, 1):
                        for it in range(2):
                            idx = 2 * parity + it
                            rr = sbCs.tile([1, 512], f32, tag="rr")
                            nc.vector.tensor_copy(
                                out=rr, in_=r4[32 * idx:32 * idx + 1, :]
                            )
                            rbc = sbCs.tile([DH, 512], f32, tag="rsbc")
                            nc.gpsimd.partition_broadcast(rbc, rr)
                            nc.vector.tensor_mul(
                                out=aoT[parity * DH:(parity + 1) * DH, q,
                                        it * 512:(it + 1) * 512],
                                in0=st["augs"][idx][0:DH, :],
                                in1=rbc,
                            )

                # head-start: q,k of pair 0 as early as the DMAs allow.
                emit_qk_chunk(0, 0, 0)
                emit_qk_chunk(0, 0, 1)
                emit_qk_chunk(0, 1, 0)
                emit_qk_chunk(0, 1, 1)

                # per-pair filler schedules: (kind, args) lists consumed one
                # slot at a time between the scores matmuls of each jt.
                v0 = [("v", jt, 0) for jt in range(NT)]
                v1 = [("v", jt, 1) for jt in range(NT)]
                qk = lambda p: [("qk", p, q, it) for q in (0, 1) for it in (0, 1)]
                # aoT[ct] is normalized at the END of pair ct+1's loop, so
                # pass-1 filler in pair p may contract up to ct = p-2.
                fillers = {
                    0: v0 + qk(1) + v1[:2],
                    1: qk(2) + v1[2:4],
                    2: qk(3) + v1[4:6],
                    3: qk(4) + v1[6:8],
                    4: qk(5) + [("p1", nt, et, 2) for nt, et in
                                ((0, 0), (0, 1), (1, 0), (1, 1))],
                    5: [("p1", nt, et, 3) for nt in range(2, 6) for et in (0, 1)],
                }

                def emit_filler(item):
                    if item[0] == "v":
                        emit_v_chunk(item[1], item[2])
                    elif item[0] == "qk":
                        emit_qk_chunk(item[1], item[2], item[3])
                    else:
                        emit_pass1_group(item[1], item[2], item[3])

                prev_st = None
                for p in range(NPAIR):
                    fill = fillers[p]
                    fi = 0
                    cur_st = {
                        "eA": sbC.tile([128, NT, N], bf16, tag="expT", name="eA"),
                        "eB": sbC.tile([128, NT, N], bf16, tag="expT", name="eB"),
                        "s4": sbCs.tile([97, 512], f32, tag="s4", name="s4"),
                        "augs": {}, "pv": {},
                    }
                    pv_slots = {1: (0, 0), 3: (0, 1), 5: (1, 0), 7: (1, 1)}
                    for jt in range(NT):
                        # 1) first half of the previous pair's PV group
                        if prev_st is not None and jt in pv_slots:
                            parity, it = pv_slots[jt]
                            emit_pv_half(p - 1, parity, it, prev_st, 0)
                        # 2) scores A: head 2p at array rows 0:64, head 2p+1
                        #    at rows 64:128 run as one concurrent row-tiled
                        #    pair per 512-column half.
                        sA = psS.tile([128, N], f32, tag="scores", name="sA")
                        for it in range(2):
                            nc.tensor.matmul(
                                sA[:, it * 512:(it + 1) * 512],
                                qkT[0:DH, 2 * p + 1, jt * 128:(jt + 1) * 128],
                                qkT[0:DH, 2 * p, it * 512:(it + 1) * 512],
                                start=True, stop=True,
                            )
                        # 3) PE cover while exp(jt-1) finishes: second half
                        #    of the PV group, or one filler chunk
                        if prev_st is not None and jt in pv_slots:
                            parity, it = pv_slots[jt]
                            emit_pv_half(p - 1, parity, it, prev_st, 1)
                        elif fi < len(fill):
                            emit_filler(fill[fi])
                            fi += 1
                        # 4) scores B
                        sB = psS.tile([128, N], f32, tag="scores", name="sB")
                        for it in range(2):
                            nc.tensor.matmul(
                                sB[:, it * 512:(it + 1) * 512],
                                qkT[DH:128, 2 * p + 1, jt * 128:(jt + 1) * 128],
                                qkT[DH:128, 2 * p, it * 512:(it + 1) * 512],
                                start=True, stop=True,
                            )
                        nc.scalar.activation(
                            out=cur_st["eA"][:, jt, :], in_=sA,
                            func=mybir.ActivationFunctionType.Exp, scale=SCALE,
                        )
                        nc.scalar.activation(
                            out=cur_st["eB"][:, jt, :], in_=sB,
                            func=mybir.ActivationFunctionType.Exp, scale=SCALE,
                        )
                        # 5) remaining filler budget for this jt
                        n_take = ((jt + 1) * len(fill)) // NT - fi
                        for _ in range(max(0, n_take)):
                            emit_filler(fill[fi])
                            fi += 1
                    if prev_st is not None:
                        emit_pv_tail(p - 1, prev_st)
                    prev_st = cur_st

                # drain: last pair's PV groups interleaved with the rest of
                # out-proj pass 1 (ct<=3 until tail(4) has run)
                for parity in (0, 1):
                    for it in range(2):
                        emit_pv_half(NPAIR - 1, parity, it, prev_st, 0)
                        emit_pv_half(NPAIR - 1, parity, it, prev_st, 1)
                        nt = 6 + 2 * parity + it
                        if nt < NT:
                            emit_pass1_group(nt, 0, 3)
                            emit_pass1_group(nt, 1, 3)
                emit_pv_tail(NPAIR - 1, prev_st)

            # ---- stage D pass 2: per (nt, et) add the remaining ct products
            # (the head pairs that weren't normalized when pass 1 ran) to the
            # pass-1 partials and store one full 3 KB row block per nt,
            # alternating store queues.
            with tc.tile_pool(name="sbDo", bufs=3) as sbDo, \
                 tc.tile_pool(name="psD2", bufs=4, space="PSUM") as psD2:
                for nt in range(NT):
                    osb = sbDo.tile([128, D], f32, tag="osb")
                    for et in range(2):
                        cts = list(range(p1_cmax[(nt, et)] + 1, KT))
                        ps = psD2.tile([128, 384], f32, tag="ops2")
                        for i, ct in enumerate(cts):
                            nc.tensor.matmul(
                                ps,
                                aoT[:, ct, nt * 128:(nt + 1) * 128],
                                wo_sb[:, ct, et * 384:(et + 1) * 384],
                                start=(i == 0), stop=(i == len(cts) - 1),
                            )
                        nc.vector.tensor_add(
                            out=osb[:, et * 384:(et + 1) * 384],
                            in0=ps,
                            in1=osb_all[:, nt, et, :],
                        )
                    eng = nc.sync if nt % 2 == 0 else nc.gpsimd
                    eng.dma_start(
                        out=out_d[nt * 128:(nt + 1) * 128, :], in_=osb
                    )
    nc.compile()
    return nc


_CACHE = {}


def _get_nc():
    if "nc" not in _CACHE:
        _CACHE["nc"] = build_bass()
    return _CACHE["nc"]


def _pack_kt(a):
    # [768, C] -> [128, 6, C] with row kt*128+p on partition p, block kt
    C = a.shape[1]
    return np.ascontiguousarray(a.reshape(KT, 128, C).transpose(1, 0, 2))


def _make_in_maps(x, w_qkv, w_out, b_out):
    bf = ml_dtypes.bfloat16
    x = np.asarray(x, dtype=np.float32)
    wq = np.asarray(w_qkv, dtype=np.float32)
    # pair-packed q|k columns: [q_p0 | k_p0 | q_p1 | k_p1 | ...]
    qk = np.empty((D, 2 * D), dtype=np.float32)
    for p in range(NPAIR):
        qk[:, 256 * p:256 * p + 128] = wq[:, 128 * p:128 * (p + 1)]
        qk[:, 256 * p + 128:256 * p + 256] = wq[:, D + 128 * p:D + 128 * (p + 1)]
    qk_pack = _pack_kt(qk).astype(bf)
    wqk1 = np.ascontiguousarray(qk_pack[:, :, 0:256])
    wqk2 = np.ascontiguousarray(qk_pack[:, :, 256:])
    wv = np.ascontiguousarray(_pack_kt(wq[:, 2 * D:]).astype(bf))
    wo = np.ascontiguousarray(_pack_kt(np.asarray(w_out, dtype=np.float32)).astype(bf))
    bo = np.ascontiguousarray(np.asarray(b_out, dtype=np.float32))
    in_maps = []
    for b in range(B):
        xT = _pack_kt(np.ascontiguousarray(x[b].T)).astype(bf)
        xa = np.ascontiguousarray(xT[:, :, 0:512])
        xb = np.ascontiguousarray(xT[:, :, 512:])
        in_maps.append({"xa": xa, "xb": xb, "wqk1": wqk1, "wqk2": wqk2,
                        "wv": wv, "wo": wo, "bo": bo})
    return in_maps


def kernel(x, w_qkv, w_out, b_out):
    nc = _get_nc()
    in_maps = _make_in_maps(x, w_qkv, w_out, b_out)
    res = run_bass_kernel_spmd(nc, in_maps, list(range(B)))
    return np.stack([res.results[b]["out"] for b in range(B)]).astype(np.float32)


# ---------------------------------------------------------------------------
# profiling helper (used by test.py only; safe no-op fallback if the axon
# NTFF hook infrastructure is unavailable)
def _install_profhook():
    import sys
    import types

    if "antenv.axon_hooks" in sys.modules:
        return True
    try:
        import antenv
        from trn_agent_boot.trn_boot import _ntff_profile_via_ctypes

        hook = _ntff_profile_via_ctypes("/opt/axon/libaxon_pjrt.so")
        mod = types.ModuleType("antenv.axon_hooks")
        mod._hook = hook
        mod.get_axon_ntff_profile_hook = lambda: mod._hook

        def _set(h):
            mod._hook = h

        mod.set_axon_ntff_profile_hook = _set
        sys.modules["antenv.axon_hooks"] = mod
        antenv.axon_hooks = mod

        import concourse.bass_utils as bu

        bu.upload_artifacts = lambda tmpdir: f"local:{tmpdir}"
        return True
    except Exception as e:  # pragma: no cover
        print(f"profhook install failed: {e}")
        return False


def run_traced(x, w_qkv, w_out, b_out, tmpdir=None):
    """Run with NTFF profiling; returns (out, exec_time_ns, results_obj)."""
    traced = _install_profhook()
    nc = _get_nc()
    in_maps = _make_in_maps(x, w_qkv, w_out, b_out)
    res = run_bass_kernel_spmd(
        nc, in_maps, list(range(B)), trace=traced, tmpdir=tmpdir
    )
    out = np.stack([res.results[b]["out"] for b in range(B)]).astype(np.float32)
    return out, res.exec_time_ns, res


# revision 13
# speedup vs baseline: 1.2244x; 1.2244x over previous
"""Multi-head attention (B=8, N=1024, D=768, H=12, softmax over full dim-scaled
scores) on 8 Trainium2 NeuronCores, data-parallel over the batch dimension:
core b computes batch element b end-to-end; no collectives.

v2 schedule (from trace analysis of the 204 us baseline):
  - Host pre-packs every tensor into its exact SBUF layout so the input
    phase is 7 large DMAs spread over 4 queues (baseline: 43 small DMAs,
    ~8 us of serialized issue).
  - The attention middle phase is ScalarE-bound (96 exp ACTs ~1.11 us each).
    Scores PSUM is one rotating 2-deep pool consumed in strict A/B
    alternation so the exp stream never waits on a PSUM bank; PV groups and
    projection/output-projection filler are interleaved between the scores
    matmuls at sub-group granularity to keep PE warm without delaying them.
  - Softmax denominators: batched reciprocal_approx_fast (51 ULP, 5x faster
    than the exact DVE reciprocal that cost 3.3 us per pair).
  - Output projection pass-1 (bias + ct0..4 of the contraction) runs as PE
    filler inside the pair-5 loop; only the ct5 rank-128 update, the final
    add and the stores remain after the drain. Stores are 8 full-row DMAs
    alternating between two queues.
"""

import numpy as np
import ml_dtypes

import concourse.bass as bass
import concourse.bacc as bacc
import concourse.tile as tile
from concourse import mybir
from concourse.bass_utils import run_bass_kernel_spmd

f32 = mybir.dt.float32
bf16 = mybir.dt.bfloat16

B = 8
N = 1024
D = 768
H = 12
DH = 64
SCALE = float(D) ** -0.5
NT = N // 128   # 8 sequence tiles
KT = D // 128   # 6 feature tiles
NPAIR = H // 2  # 6 head pairs


def build_bass():
    nc = bacc.Bacc("TRN2", target_bir_lowering=False, debug=False, num_devices=B)
    # all inputs pre-packed on host to [partition, kt, col] SBUF layouts;
    # each DMA's SBUF destination is fully contiguous (strided destinations
    # fragment the transfer into ~1 KB packets and halve queue bandwidth)
    xa_d = nc.dram_tensor("xa", [128, 3, N], bf16, kind="ExternalInput")
    xb_d = nc.dram_tensor("xb", [128, 3, N], bf16, kind="ExternalInput")
    wqk0_d = nc.dram_tensor("wqk0", [128, KT, 256], bf16, kind="ExternalInput")
    wqkra_d = nc.dram_tensor("wqkra", [128, 3, 1280], bf16, kind="ExternalInput")
    wqkrb_d = nc.dram_tensor("wqkrb", [128, 3, 1280], bf16, kind="ExternalInput")
    wv_d = nc.dram_tensor("wv", [128, KT, D], bf16, kind="ExternalInput")
    wo_d = nc.dram_tensor("wo", [128, KT, D], bf16, kind="ExternalInput")
    bo_d = nc.dram_tensor("bo", [D], f32, kind="ExternalInput")
    out_d = nc.dram_tensor("out", [N, D], f32, kind="ExternalOutput")

    with tile.TileContext(nc) as tc:
        with tc.tile_pool(name="persist", bufs=1) as pp:
            # persistent SBUF tensors
            xT_sb = pp.tile([128, KT, N], bf16)        # x^T feature tiles
            wqk0_sb = pp.tile([128, KT, 256], bf16)    # pair-0 q|k cols
            wqkr_sb = pp.tile([128, KT, 1280], bf16)   # pair 1-5 q|k cols
            wv_sb = pp.tile([128, KT, D], bf16)        # v cols (head order)
            wo_sb = pp.tile([128, KT, D], bf16)        # W_out feature tiles
            qkT = pp.tile([128, 2 * KT, N], bf16)      # q,k feature-major
            vaug = pp.tile([128, NT, H, DH + 1], bf16)  # v token-major + ones
            aoT = pp.tile([128, KT, N], bf16)          # attention out, feature-major
            osb_all = pp.tile([128, NT, 2, 384], bf16)  # out-proj pass-1 partials
            bias_f32 = pp.tile([1, D], f32)
            bias_bf = pp.tile([1, D], bf16)
            ones_col = pp.tile([1, 128], bf16)

            # ---- input DMAs: contiguous destinations, three queues, ordered
            # by first use (x + pair-0 weights gate the head start).
            nc.sync.dma_start(out=xT_sb[:, 0:3, :], in_=xa_d[:, :, :])
            nc.scalar.dma_start(out=xT_sb[:, 3:6, :], in_=xb_d[:, :, :])
            nc.gpsimd.dma_start(out=wqk0_sb, in_=wqk0_d[:, :, :])
            nc.gpsimd.dma_start(out=wv_sb, in_=wv_d[:, :, :])
            nc.sync.dma_start(out=wqkr_sb[:, 0:3, :], in_=wqkra_d[:, :, :])
            nc.scalar.dma_start(out=wqkr_sb[:, 3:6, :], in_=wqkrb_d[:, :, :])
            nc.gpsimd.dma_start(out=wo_sb, in_=wo_d[:, :, :])
            bo_ap = bo_d[:]
            nc.sync.dma_start(
                out=bias_f32,
                in_=bass.AP(tensor=bo_ap.tensor, offset=bo_ap.offset,
                            ap=[[0, 1]] + list(bo_ap.ap)),
            )
            nc.vector.memset(vaug[:, :, :, DH], 1.0)
            warm = pp.tile([1, 2], f32)
            nc.vector.memset(warm, 0.0)
            nc.vector.tensor_copy(out=bias_bf, in_=bias_f32)
            nc.gpsimd.memset(ones_col, 1.0)
            # dummy activation: pulls the exp ACT-table load (~2.7 us) into
            # the initial DMA wait instead of the first real exp
            nc.scalar.activation(out=warm, in_=warm,
                                 func=mybir.ActivationFunctionType.Exp)

            with tc.tile_pool(name="sbC", bufs=4) as sbC, \
                 tc.tile_pool(name="sbAug", bufs=5) as sbAug, \
                 tc.tile_pool(name="sbCs", bufs=2) as sbCs, \
                 tc.tile_pool(name="psS", bufs=3, space="PSUM") as psS, \
                 tc.tile_pool(name="psF", bufs=2, space="PSUM") as psF:

                # ~4 us of dummy matmuls on garbage SBUF: keeps the PE busy
                # during the input-DMA wait so HAM releases the clock gate
                # (1.2 -> 2.4 GHz) before the first real matmul.
                wm = psF.tile([128, 512], f32, tag="fill", name="wm")
                for _ in range(10):
                    nc.tensor.matmul(wm, aoT[:, 0, 0:128], aoT[:, 0, 0:512],
                                     start=True, stop=True)

                def emit_qk_chunk(p, qk, it):
                    # qkT[:, 2p+qk, it-half] = W_{q|k,pair p}^T @ x^T
                    ps = psF.tile([128, 512], f32, tag="fill", name="psqk")
                    for kt in range(KT):
                        if p == 0:
                            w = wqk0_sb[:, kt, 128 * qk:128 * (qk + 1)]
                        else:
                            w = wqkr_sb[:, kt, 256 * (p - 1) + 128 * qk:
                                        256 * (p - 1) + 128 * (qk + 1)]
                        nc.tensor.matmul(
                            ps, w,
                            xT_sb[:, kt, it * 512:(it + 1) * 512],
                            start=(kt == 0), stop=(kt == KT - 1),
                        )
                    nc.vector.tensor_copy(
                        out=qkT[:, 2 * p + qk, it * 512:(it + 1) * 512], in_=ps
                    )

                def emit_v_chunk(jt, et):
                    # v[jt-tile, 6 heads] = x @ W_v  (+ strided head layout)
                    ps = psF.tile([128, 384], f32, tag="fill", name="psv")
                    for kt in range(KT):
                        nc.tensor.matmul(
                            ps,
                            xT_sb[:, kt, jt * 128:(jt + 1) * 128],
                            wv_sb[:, kt, et * 384:(et + 1) * 384],
                            start=(kt == 0), stop=(kt == KT - 1),
                        )
                    nc.vector.tensor_copy(
                        out=vaug[:, jt, 6 * et:6 * (et + 1), 0:DH],
                        in_=ps.rearrange("p (h d) -> p h d", d=DH),
                    )

                p1_cmax = {}

                def emit_pass1_group(nt, et, cmax):
                    # osb_all[nt, et] = bias + sum_{ct<=cmax} aoT_ct^T @ W_out
                    # (cmax limited by which head pairs are normalized yet;
                    # pass 2 adds the rest)
                    p1_cmax[(nt, et)] = cmax
                    ps = psF.tile([128, 384], f32, tag="fill", name="psp1")
                    nc.tensor.matmul(
                        ps, ones_col, bias_bf[:, et * 384:(et + 1) * 384],
                        start=True, stop=False,
                    )
                    for ct in range(cmax + 1):
                        nc.tensor.matmul(
                            ps,
                            aoT[:, ct, nt * 128:(nt + 1) * 128],
                            wo_sb[:, ct, et * 384:(et + 1) * 384],
                            start=False, stop=(ct == cmax),
                        )
                    nc.vector.tensor_copy(out=osb_all[:, nt, et, :], in_=ps)

                def emit_pv_group(q, parity, it, st):
                    # softmax-numerator matmul group of pair q:
                    # out_augT[d+1, i-half] = [v_h | 1]^T @ expT_h
                    e = st["eA"] if parity == 0 else st["eB"]
                    h = 2 * q + parity
                    idx = 2 * parity + it
                    ops = psF.tile([DH + 1, 512], f32, tag="fill", name="pvps")
                    for jt_ in range(NT):
                        nc.tensor.matmul(
                            ops,
                            vaug[:, jt_, h, :],
                            e[:, jt_, it * 512:(it + 1) * 512],
                            start=(jt_ == 0), stop=(jt_ == NT - 1),
                        )
                    aug = sbAug.tile([DH + 1, 512], f32, tag="aug")
                    nc.vector.tensor_copy(out=aug, in_=ops)
                    nc.vector.tensor_copy(
                        out=st["s4"][32 * idx:32 * idx + 1, :],
                        in_=aug[DH:DH + 1, :],
                    )
                    st["augs"][idx] = aug

                def emit_pv_tail(q, st):
                    # batched approx-reciprocal of the pair's 4 softmax
                    # denominators (rows 0/32/64/96 of s4), broadcast, scale.
                    r4 = sbCs.tile([97, 512], f32, tag="r4")
                    nc.vector.reciprocal_approx_fast(out=r4, in_=st["s4"])
                    for parity in (0, 1):
                        for it in range(2):
                            idx = 2 * parity + it
                            rr = sbCs.tile([1, 512], f32, tag="rr")
                            nc.vector.tensor_copy(
                                out=rr, in_=r4[32 * idx:32 * idx + 1, :]
                            )
                            rbc = sbCs.tile([DH, 512], f32, tag="rsbc")
                            nc.gpsimd.partition_broadcast(rbc, rr)
                            nc.vector.tensor_mul(
                                out=aoT[parity * DH:(parity + 1) * DH, q,
                                        it * 512:(it + 1) * 512],
                                in0=st["augs"][idx][0:DH, :],
                                in1=rbc,
                            )

                # head-start: q,k of pair 0 as early as the DMAs allow.
                emit_qk_chunk(0, 0, 0)
                emit_qk_chunk(0, 0, 1)
                emit_qk_chunk(0, 1, 0)
                emit_qk_chunk(0, 1, 1)

                # per-pair filler schedules: (kind, args) lists consumed one
                # slot at a time between the scores matmuls of each jt.
                v0 = [("v", jt, 0) for jt in range(NT)]
                v1 = [("v", jt, 1) for jt in range(NT)]
                qk = lambda p: [("qk", p, q, it) for q in (0, 1) for it in (0, 1)]
                # aoT[ct] is normalized at the END of pair ct+1's loop, so
                # pass-1 filler in pair p may contract up to ct = p-2.
                fillers = {
                    0: v0 + v1[:2] + qk(1),
                    1: qk(2) + v1[2:4],
                    2: qk(3) + v1[4:6],
                    3: qk(4) + v1[6:8],
                    4: qk(5) + [("p1", nt, et, 2) for nt, et in
                                ((0, 0), (0, 1), (1, 0), (1, 1))],
                    5: [("p1", nt, et, 3) for nt in range(2, 6) for et in (0, 1)],
                }

                def emit_filler(item):
                    if item[0] == "v":
                        emit_v_chunk(item[1], item[2])
                    elif item[0] == "qk":
                        emit_qk_chunk(item[1], item[2], item[3])
                    else:
                        emit_pass1_group(item[1], item[2], item[3])

                prev_st = None
                for p in range(NPAIR):
                    fill = fillers[p]
                    fi = 0
                    cur_st = {
                        "eA": sbC.tile([128, NT, N], bf16, tag="expT", name="eA"),
                        "eB": sbC.tile([128, NT, N], bf16, tag="expT", name="eB"),
                        "s4": sbCs.tile([97, 512], f32, tag="s4", name="s4"),
                        "augs": {},
                    }
                    pv_slots = {1: (0, 0), 3: (0, 1), 5: (1, 0), 7: (1, 1)}
                    for jt in range(NT):
                        # scores A/B adjacent: head 2p at array rows 0:64 and
                        # head 2p+1 at rows 64:128 run as concurrent
                        # row-tiled pairs; the 3-deep psS rotation means the
                        # destination banks freed >=2 exp periods ago, so
                        # these never stall and the exp stream stays gapless.
                        sA = psS.tile([128, N], f32, tag="scores", name="sA")
                        sB = psS.tile([128, N], f32, tag="scores", name="sB")
                        for it in range(2):
                            nc.tensor.matmul(
                                sA[:, it * 512:(it + 1) * 512],
                                qkT[0:DH, 2 * p + 1, jt * 128:(jt + 1) * 128],
                                qkT[0:DH, 2 * p, it * 512:(it + 1) * 512],
                                start=True, stop=True,
                            )
                        for it in range(2):
                            nc.tensor.matmul(
                                sB[:, it * 512:(it + 1) * 512],
                                qkT[DH:128, 2 * p + 1, jt * 128:(jt + 1) * 128],
                                qkT[DH:128, 2 * p, it * 512:(it + 1) * 512],
                                start=True, stop=True,
                            )
                        nc.scalar.activation(
                            out=cur_st["eA"][:, jt, :], in_=sA,
                            func=mybir.ActivationFunctionType.Exp, scale=SCALE,
                        )
                        nc.scalar.activation(
                            out=cur_st["eB"][:, jt, :], in_=sB,
                            func=mybir.ActivationFunctionType.Exp, scale=SCALE,
                        )
                        # previous pair's PV group, then filler, as PE cover
                        if prev_st is not None and jt in pv_slots:
                            parity, it = pv_slots[jt]
                            emit_pv_group(p - 1, parity, it, prev_st)
                        n_take = ((jt + 1) * len(fill)) // NT - fi
                        for _ in range(max(0, n_take)):
                            emit_filler(fill[fi])
                            fi += 1
                    if prev_st is not None:
                        emit_pv_tail(p - 1, prev_st)
                    prev_st = cur_st

                # drain: last pair's PV groups + final out-proj pass-1 groups
                # (tail(4) has run, so ct<=4 is available for nt 6..7)
                for parity in (0, 1):
                    for it in range(2):
                        emit_pv_group(NPAIR - 1, parity, it, prev_st)
                        nt = 6 + 2 * parity + it
                        if nt < NT:
                            emit_pass1_group(nt, 0, 4)
                            emit_pass1_group(nt, 1, 4)
                emit_pv_tail(NPAIR - 1, prev_st)

            # ---- stage D pass 2: per (nt, et) add the remaining ct products
            # (the head pairs that weren't normalized when pass 1 ran) to the
            # pass-1 partials and store one full 3 KB row block per nt,
            # alternating store queues.
            with tc.tile_pool(name="sbDo", bufs=3) as sbDo, \
                 tc.tile_pool(name="psD2", bufs=4, space="PSUM") as psD2:
                # nt 6,7 (single ct5 matmul) first: they absorb the one
                # unavoidable wait on the last pair's normalization; the
                # deeper ct3..5 groups for nt 0,1 then run stall-free.
                for nt in (6, 7, 0, 1, 2, 3, 4, 5):
                    osb = sbDo.tile([128, D], f32, tag="osb")
                    for et in range(2):
                        cts = list(range(p1_cmax[(nt, et)] + 1, KT))
                        ps = psD2.tile([128, 384], f32, tag="ops2")
                        for i, ct in enumerate(cts):
                            nc.tensor.matmul(
                                ps,
                                aoT[:, ct, nt * 128:(nt + 1) * 128],
                                wo_sb[:, ct, et * 384:(et + 1) * 384],
                                start=(i == 0), stop=(i == len(cts) - 1),
                            )
                        nc.vector.tensor_add(
                            out=osb[:, et * 384:(et + 1) * 384],
                            in0=ps,
                            in1=osb_all[:, nt, et, :],
                        )
                    eng = nc.sync if nt % 2 == 0 else nc.gpsimd
                    eng.dma_start(
                        out=out_d[nt * 128:(nt + 1) * 128, :], in_=osb
                    )
    nc.compile()
    return nc


_CACHE = {}


def _get_nc():
    if "nc" not in _CACHE:
        _CACHE["nc"] = build_bass()
    return _CACHE["nc"]


def _pack_kt(a):
    # [768, C] -> [128, 6, C] with row kt*128+p on partition p, block kt
    C = a.shape[1]
    return np.ascontiguousarray(a.reshape(KT, 128, C).transpose(1, 0, 2))


def _make_in_maps(x, w_qkv, w_out, b_out):
    bf = ml_dtypes.bfloat16
    x = np.asarray(x, dtype=np.float32)
    wq = np.asarray(w_qkv, dtype=np.float32)
    # pair-packed q|k columns: [q_p0 | k_p0 | q_p1 | k_p1 | ...]
    qk = np.empty((D, 2 * D), dtype=np.float32)
    for p in range(NPAIR):
        qk[:, 256 * p:256 * p + 128] = wq[:, 128 * p:128 * (p + 1)]
        qk[:, 256 * p + 128:256 * p + 256] = wq[:, D + 128 * p:D + 128 * (p + 1)]
    qk_pack = _pack_kt(qk).astype(bf)
    wqk0 = np.ascontiguousarray(qk_pack[:, :, 0:256])
    wqkr = qk_pack[:, :, 256:]
    wqkra = np.ascontiguousarray(wqkr[:, 0:3, :])
    wqkrb = np.ascontiguousarray(wqkr[:, 3:6, :])
    wv = np.ascontiguousarray(_pack_kt(wq[:, 2 * D:]).astype(bf))
    wo = np.ascontiguousarray(_pack_kt(np.asarray(w_out, dtype=np.float32)).astype(bf))
    bo = np.ascontiguousarray(np.asarray(b_out, dtype=np.float32))
    in_maps = []
    for b in range(B):
        xT = _pack_kt(np.ascontiguousarray(x[b].T)).astype(bf)
        xa = np.ascontiguousarray(xT[:, 0:3, :])
        xb = np.ascontiguousarray(xT[:, 3:6, :])
        in_maps.append({"xa": xa, "xb": xb, "wqk0": wqk0, "wqkra": wqkra,
                        "wqkrb": wqkrb, "wv": wv, "wo": wo, "bo": bo})
    return in_maps


def kernel(x, w_qkv, w_out, b_out):
    nc = _get_nc()
    in_maps = _make_in_maps(x, w_qkv, w_out, b_out)
    res = run_bass_kernel_spmd(nc, in_maps, list(range(B)))
    return np.stack([res.results[b]["out"] for b in range(B)]).astype(np.float32)


# ---------------------------------------------------------------------------
# profiling helper (used by test.py only; safe no-op fallback if the axon
# NTFF hook infrastructure is unavailable)
def _install_profhook():
    import sys
    import types

    if "antenv.axon_hooks" in sys.modules:
        return True
    try:
        import antenv
        from trn_agent_boot.trn_boot import _ntff_profile_via_ctypes

        hook = _ntff_profile_via_ctypes("/opt/axon/libaxon_pjrt.so")
        mod = types.ModuleType("antenv.axon_hooks")
        mod._hook = hook
        mod.get_axon_ntff_profile_hook = lambda: mod._hook

        def _set(h):
            mod._hook = h

        mod.set_axon_ntff_profile_hook = _set
        sys.modules["antenv.axon_hooks"] = mod
        antenv.axon_hooks = mod

        import concourse.bass_utils as bu

        bu.upload_artifacts = lambda tmpdir: f"local:{tmpdir}"
        return True
    except Exception as e:  # pragma: no cover
        print(f"profhook install failed: {e}")
        return False


def run_traced(x, w_qkv, w_out, b_out, tmpdir=None):
    """Run with NTFF profiling; returns (out, exec_time_ns, results_obj)."""
    traced = _install_profhook()
    nc = _get_nc()
    in_maps = _make_in_maps(x, w_qkv, w_out, b_out)
    res = run_bass_kernel_spmd(
        nc, in_maps, list(range(B)), trace=traced, tmpdir=tmpdir
    )
    out = np.stack([res.results[b]["out"] for b in range(B)]).astype(np.float32)
    return out, res.exec_time_ns, res
